# revision 1
# baseline (speedup 1.0000x reference)
"""ArcFace loss on 8 TRN2 NeuronCores — class-dimension (C) sharded.

Math (reference has M1=1, M2=0.5, M3=0, scale=64, label_smoothing=0):
  per row i with one-hot y_true:  v_i = x[i, label_i] = sum_j y[i,j]*x[i,j]
  t_i = cos(acos(v_i) + 0.5),  t_i -> -2 - t_i when v_i <= cos(pi - 0.5)
  loss_i = logsumexp_j(64 * modified_x[i,j]) - 64*t_i
  loss = mean_i loss_i          (0 when a row of y_true is all zero)

All logits lie in (-0.99, 0.99), so 64*x - 64 <= 0 and a FIXED shift of 64
replaces the row-max in logsumexp (no max pass, no second streaming pass):
  logsumexp_i = 64 + log(S_i),
  S_i = sum_j exp(64*x[i,j] - 64) + exp(64*t_i - 64) - exp(64*v_i - 64)

Each core streams its [512, 12500] shard of x (f32) and y (staged as uint8 —
lossless for an exact {0,1} one-hot, and 4x fewer bytes) once and emits
per-row partials:
  hvh_i = sum_j (x[i,j] + 16) * y[i,j]   (= v_i + 16 if the label is local,
                                          exactly 0 otherwise — encodes both
                                          the hit flag and the hit value)
  se_i  = sum_j exp(64*x[i,j] - 64)
plus column 0 of the local shard (needed to mimic argmax(all-zeros)=0 when a
y_true row is entirely zero — the reference then returns a 0 contribution,
so col0 is only used to keep the formulas well-defined).

The host "unshard" step sums the [512]-sized partials over the 8 cores and
applies the closed-form tail (acos/cos/log on 512 scalars).
"""

import os

import numpy as np

B = 512
C = 100000
NCORES = 8
CS = C // NCORES  # 12500 classes per core
P = 128
RG = B // P  # 4 row groups of 128 partitions
FCH = int(os.environ.get("AK_FCH", "6250"))  # free-dim chunk
NCH = CS // FCH  # chunks per row group
XBUFS = int(os.environ.get("AK_XBUFS", "2"))
YBUFS = int(os.environ.get("AK_YBUFS", "2"))
EBUFS = int(os.environ.get("AK_EBUFS", "2"))
YENG = os.environ.get("AK_YENG", "sync")  # engine issuing y-shard loads
EOUT = os.environ.get("AK_EOUT", "scratch")  # exp 'out' target: scratch|dummy|inplace
# y_true is an exact {0.0, 1.0} one-hot, so staging it as uint8 is lossless
# (the DVE converts u8 -> fp32 0/1 in-datapath; results are bit-identical to
# f32-staged y in every measured run) and cuts the streamed bytes from
# 51.2 MB to 32 MB per core.  x stays f32 for full precision; "bf16" staging
# of x is supported (another 1.35x, measured rel err ~7e-5) but off by default.
YDTYPE = os.environ.get("AK_YDTYPE", "u8")  # y staging dtype: f32|u8|u8cast
XDTYPE = os.environ.get("AK_XDTYPE", "f32")  # x staging dtype: f32|bf16
YFCH = int(os.environ.get("AK_YFCH", str(FCH)))  # y free-dim chunk (multiple of FCH)
assert YFCH % FCH == 0 and CS % YFCH == 0
TAILSPLIT = os.environ.get("AK_TAILSPLIT", "1") == "1"  # halve the final chunk twice
# stage shards host-side as [RG*NCH*P, FCH] so each [128, FCH] tile is one
# fully-contiguous DRAM block (the plain [512, 12500] layout makes every tile
# DMA a 128-row strided read, which sustains only ~324 GB/s of the ~358 peak)
CONTIG = os.environ.get("AK_CONTIG", "0") == "1"
HEADSPLIT = os.environ.get("AK_HEADSPLIT", "0") == "1"
POOLMODE = os.environ.get("AK_POOLMODE", "stack")  # TileContext pool_alloc_mode
# 3-step taper of the final chunk: the kernel's exit is bound by the last
# out-DMA's completion receipt, so shrinking the final compute tail moves the
# out trigger (and the whole kernel end) earlier
TAPER = os.environ.get("AK_TAPER", "1") == "1"
OENG = os.environ.get("AK_OENG", "sync")  # engine issuing the output DMA
# issue each x-tile as two half-DMAs on the two HWDGE rings (SP + ACT)
XSPLITRING = os.environ.get("AK_XSPLITRING", "0") == "1"

KOFF = 16.0  # hit-encoding offset: hvh = v + 16 iff label is in-shard
SCALE = 64.0
M2 = 0.5
THRESHOLD = float(np.cos(np.pi - M2))

_CACHE = {}


def _build_nc():
    import concourse.tile as tile
    from concourse import bacc, mybir

    nc = bacc.Bacc(
        "TRN2",
        target_bir_lowering=False,
        debug=False,
        enable_asserts=False,
        num_devices=NCORES,
    )
    f32 = mybir.dt.float32
    y_dt = f32 if YDTYPE == "f32" else mybir.dt.uint8
    x_dt = f32 if XDTYPE == "f32" else mybir.dt.bfloat16
    if CONTIG:
        assert YFCH == FCH
        x_d = nc.dram_tensor("x", [RG * NCH * P, FCH], x_dt, kind="ExternalInput").ap()
        y_d = nc.dram_tensor("y", [RG * NCH * P, FCH], y_dt, kind="ExternalInput").ap()
    else:
        x_d = nc.dram_tensor("x", [B, CS], x_dt, kind="ExternalInput").ap()
        y_d = nc.dram_tensor("y", [B, CS], y_dt, kind="ExternalInput").ap()
    # out columns: [0:RG] hvh per row group, [RG:2RG] se, [2RG:3RG] shard col0
    out_d = nc.dram_tensor("out", [P, 3 * RG], f32, kind="ExternalOutput").ap()

    with tile.TileContext(nc, pool_alloc_mode=POOLMODE) as tc:
        with (
            tc.tile_pool(name="xin", bufs=XBUFS) as xpool,
            tc.tile_pool(name="yin", bufs=YBUFS) as ypool,
            tc.tile_pool(name="escratch", bufs=EBUFS) as epool,
            tc.tile_pool(name="stats", bufs=1) as stats,
        ):
            y_dma = getattr(nc, YENG)
            hvh_parts = stats.tile([P, RG * NCH + 2], f32)
            se_parts = stats.tile([P, RG * NCH + 2], f32)
            outsb = stats.tile([P, 3 * RG], f32)
            dummy = stats.tile([P, 1], f32)
            dummy2 = stats.tile([P, 1], f32)
            neg_scale = stats.tile([P, 1], f32)
            nc.vector.memset(neg_scale[:], -SCALE)

            yt_dt = f32 if YDTYPE in ("f32", "u8cast") else mybir.dt.uint8
            y_loader = nc.gpsimd if YDTYPE == "u8cast" else y_dma
            i = 0  # global partial-column index
            for r in range(RG):
                widths = [FCH] * NCH
                if TAILSPLIT and r == RG - 1:
                    # shrink the final chunks so less compute trails the last DMA
                    if TAPER:
                        h1 = FCH // 2
                        h2 = (FCH - h1) - (FCH - h1) // 2
                        h3 = (FCH - h1) - h2
                        widths = [FCH] * (NCH - 1) + [h1, h2, h3]
                    else:
                        widths = [FCH] * (NCH - 1) + [FCH - FCH // 2, FCH // 2]
                if HEADSPLIT and r == 0:
                    # small first chunk: compute starts while the prefill drains
                    widths = [FCH // 2, FCH - FCH // 2] + widths[1:]
                i0, off, yt, ybase = i, 0, None, -1
                for w in widths:
                    if CONTIG:
                        blk = (r * NCH + off // FCH) * P
                        x_src = x_d[blk : blk + P, off % FCH : off % FCH + w]
                    else:
                        x_src = x_d[r * P : (r + 1) * P, off : off + w]
                    xt = xpool.tile([P, FCH], x_dt, tag="xt")
                    if XSPLITRING and w > 1:
                        h = w // 2
                        nc.sync.dma_start(xt[:, :h], x_src[:, :h])
                        nc.scalar.dma_start(xt[:, h:w], x_src[:, h:])
                    else:
                        nc.sync.dma_start(xt[:, :w], x_src)
                    if off // YFCH != ybase:
                        ybase = off // YFCH
                        yt = ypool.tile([P, YFCH], yt_dt, tag="yt")
                        if CONTIG:
                            yblk = (r * NCH + ybase) * P
                            y_src = y_d[yblk : yblk + P, :]
                        else:
                            y_src = y_d[
                                r * P : (r + 1) * P,
                                ybase * YFCH : (ybase + 1) * YFCH,
                            ]
                        # u8cast: SWDGE casts u8->f32 during the DMA itself
                        y_loader.dma_start(yt[:], y_src)
                    yc = off - ybase * YFCH
                    # DVE: hvh partial = sum((x + 16) * y) along the chunk
                    nc.vector.scalar_tensor_tensor(
                        out=dummy.broadcast_to([P, w]),
                        in0=xt[:, :w],
                        scalar=KOFF,
                        in1=yt[:, yc : yc + w],
                        op0=mybir.AluOpType.add,
                        op1=mybir.AluOpType.mult,
                        accum_out=hvh_parts[:, i : i + 1],
                    )
                    # ACT: se partial = sum(exp(64*x - 64)) along the chunk
                    if EOUT == "dummy":
                        et_ap = dummy2.broadcast_to([P, w])
                    elif EOUT == "inplace":
                        et_ap = xt[:, :w]
                    else:
                        et = epool.tile([P, FCH], f32, tag="et")
                        et_ap = et[:, :w]
                    nc.scalar.activation(
                        out=et_ap,
                        in_=xt[:, :w],
                        func=mybir.ActivationFunctionType.Exp,
                        bias=neg_scale[:],
                        scale=SCALE,
                        accum_out=se_parts[:, i : i + 1],
                    )
                    if off == 0:
                        nc.vector.tensor_copy(
                            outsb[:, 2 * RG + r : 2 * RG + r + 1], xt[:, 0:1]
                        )
                    off += w
                    i += 1
                # per-group combine right after the group's chunks
                nc.vector.tensor_reduce(
                    out=outsb[:, r : r + 1],
                    in_=hvh_parts[:, i0:i],
                    axis=mybir.AxisListType.X,
                    op=mybir.AluOpType.add,
                )
                nc.vector.tensor_reduce(
                    out=outsb[:, RG + r : RG + r + 1],
                    in_=se_parts[:, i0:i],
                    axis=mybir.AxisListType.X,
                    op=mybir.AluOpType.add,
                )
            getattr(nc, OENG).dma_start(out_d[:], outsb[:])

    nc.compile()
    return nc


def _get_nc():
    if "nc" not in _CACHE:
        _CACHE["nc"] = _build_nc()
    return _CACHE["nc"]


def _run_device(y_true, norm_logits, trace=False, trace_cores=None):
    from concourse import bass_utils

    nc = _get_nc()
    x = np.ascontiguousarray(np.asarray(norm_logits, dtype=np.float32))
    y = np.ascontiguousarray(np.asarray(y_true, dtype=np.float32))
    y_np = np.float32 if YDTYPE == "f32" else np.uint8
    if XDTYPE == "f32":
        x_np = np.float32
    else:
        import ml_dtypes

        x_np = ml_dtypes.bfloat16
    def stage(a, dt):
        shards = []
        for k in range(NCORES):
            s = a[:, k * CS : (k + 1) * CS].astype(dt)
            if CONTIG:
                # [512, 12500] -> [RG*NCH*P, FCH]: each [128, FCH] tile becomes
                # one contiguous DRAM block
                s = (
                    s.reshape(RG, P, NCH, FCH)
                    .transpose(0, 2, 1, 3)
                    .reshape(RG * NCH * P, FCH)
                )
            shards.append(np.ascontiguousarray(s))
        return shards

    xs, ys = stage(x, x_np), stage(y, y_np)
    in_maps = [{"x": xs[k], "y": ys[k]} for k in range(NCORES)]
    kwargs = {}
    if trace:
        kwargs["trace"] = True
        kwargs["trace_cores"] = (
            list(range(NCORES)) if trace_cores is None else trace_cores
        )
    return bass_utils.run_bass_kernel_spmd(
        nc, in_maps, core_ids=list(range(NCORES)), **kwargs
    )


def _combine(core_outs):
    """Unshard: sum per-core [128, 12] partials and apply the scalar tail."""
    arr = np.stack([np.asarray(o, dtype=np.float64) for o in core_outs])  # [8,128,12]
    # column p of row group r holds global row r*128 + p -> transpose to [RG, P]
    hvh = arr[:, :, 0:RG].sum(axis=0).T.reshape(-1)  # [512]
    se = arr[:, :, RG : 2 * RG].sum(axis=0).T.reshape(-1)  # [512]
    col0 = arr[0, :, 2 * RG : 3 * RG].T.reshape(-1)  # [512] (global col 0 = core 0)

    hit = hvh > KOFF / 2  # exactly one hit: hvh = v + 16 in [15.01, 16.99]
    v = np.where(hit, hvh - KOFF, col0)
    t = np.cos(np.arccos(np.clip(v, -1.0, 1.0)) + M2)
    tv = np.where(v > THRESHOLD, t, -2.0 - t)
    S = se + hit * (np.exp(SCALE * tv - SCALE) - np.exp(SCALE * v - SCALE))
    loss_rows = hit * (SCALE + np.log(S) - SCALE * tv)
    return np.asarray(loss_rows.mean(), dtype=np.float32)


def kernel(y_true, norm_logits):
    res = _run_device(y_true, norm_logits)
    return _combine([r["out"] for r in res.results])



# revision 5
# speedup vs baseline: 1.5137x; 1.5137x over previous
"""ArcFace loss on 8 TRN2 NeuronCores — class-dimension (C) sharded, v2.

Math (reference has M1=1, M2=0.5, M3=0, scale=64, label_smoothing=0):
  per row i with one-hot y_true:  v_i = x[i, label_i]
  t_i = cos(acos(v_i) + 0.5),  t_i -> -2 - t_i when v_i <= cos(pi - 0.5)
  loss_i = logsumexp_j(64 * modified_x[i,j]) - 64*t_i   (0 for all-zero rows)

All logits lie in (-0.99, 0.99) so a FIXED shift of 64 replaces the row max:
  logsumexp_i = 64 + log(S_i),  S_i = sum_j exp(64*x[i,j] - 64) (+ hit swap)

v2 design (vs v1 which streamed y as u8 and burned a full DVE pass on
sum((x+16)*y)): y is an exact one-hot, so the host re-encodes it losslessly
as per-row gather indices + a tiny select mask; the device gathers its local
label hits from its own streamed x data (gpsimd.ap_gather over the
SBUF-resident shard — the "gather local label hits" step of the partial-FC
sharding hint).  This removes the 6.4 MB/core y stream and frees the DVE.

Per core (shard [512, 12500] of x, staged bf16 = 12.8 MB):
  - x chunks DMA into 4 SBUF-resident row-group tiles [128, 12500]
  - ACT: se partials = sum exp(64x - 64) via activation(Exp, accum_out)
  - DVE (SCH>0): the last SCH columns of each row group instead use a
    Schraudolph exp: bits = u16(round(A*x + B)) saturating (<0 -> 0 = +0.0
    in bf16), then tensor_reduce over the u16 tile bitcast to bf16.  The
    bit pattern of 2^u is 128*(u+127) for the bf16 grid, so sum of
    bitcast-bf16 values approximates the exp-sum (few-% per-term error,
    ~1e-5 on the final loss).  This splits the elementwise-exp work across
    ACT and the otherwise idle DVE.
  - GPSIMD: ap_gather pulls 16 candidate hit pairs per 16-partition core
    from each resident row group; a [128, 32] host-staged one-hot mask +
    stt(accum_out) extracts v_i (0 when the label is not in this shard).

Host combine: sum the [512] partials over 8 cores, apply the scalar tail
(acos/cos/log on <=512 values) in float64 — same split as v1.
"""

import os

import numpy as np

B = 512
C = 100000
NCORES = 8
CS = C // NCORES  # 12500 classes per core
P = 128
RG = B // P  # 4 row groups
SCH = int(os.environ.get("AK_SCH", "0"))  # Schraudolph cols per row group
FCH = int(os.environ.get("AK_FCH", "6250"))  # ACT chunk width
SFCH = int(os.environ.get("AK_SFCH", "3584"))  # Schraudolph chunk width
CONTIG = os.environ.get("AK_CONTIG", "0") == "1"
EBUFS = int(os.environ.get("AK_EBUFS", "2"))
XDT = os.environ.get("AK_XDT", "bf16")
GENG = os.environ.get("AK_GENG", "scalar")  # ring for gi/msk loads
OENG = os.environ.get("AK_OENG", "sync")  # ring for the output store

SCALE = 64.0
M2 = 0.5
THRESHOLD = float(np.cos(np.pi - M2))
LOG2E = float(np.log2(np.e))
SCH_A = 128.0 * SCALE * LOG2E  # 11818.6...
# bits(2^u) ~= 128*(u+127); u = 64*log2e*(x-1); -128*0.0287 zeroes the mean
# multiplicative bias of the linear-mantissa approximation
SCH_B = 128.0 * (127.0 - SCALE * LOG2E) - 128.0 * 0.0287

_CACHE = {}


def _chunks(total, step):
    out, off = [], 0
    while off < total:
        w = min(step, total - off)
        out.append((off, w))
        off += w
    return out


def _build_nc():
    import concourse.tile as tile
    from concourse import bacc, mybir

    nc = bacc.Bacc(
        "TRN2",
        target_bir_lowering=False,
        debug=False,
        enable_asserts=False,
        num_devices=NCORES,
    )
    f32 = mybir.dt.float32
    bf16 = mybir.dt.bfloat16
    i16 = mybir.dt.int16
    u16 = mybir.dt.uint16
    x_dt = bf16 if XDT == "bf16" else f32

    ACT_W = CS - SCH  # ACT-exp columns per row group
    x_d = nc.dram_tensor("x", [B, CS], x_dt, kind="ExternalInput").ap()
    gi_d = nc.dram_tensor("gi", [P, RG], i16, kind="ExternalInput").ap()
    msk_d = nc.dram_tensor("msk", [P, RG * 32], bf16, kind="ExternalInput").ap()
    out_d = nc.dram_tensor("out", [P, 2 * RG], f32, kind="ExternalOutput").ap()

    act_chunks = _chunks(ACT_W, FCH)
    sch_chunks = _chunks(SCH, SFCH)
    NSE = len(act_chunks) + len(sch_chunks)  # se partial cols per row group

    with tile.TileContext(nc) as tc:
        with (
            tc.tile_pool(name="res", bufs=1) as res,
            tc.tile_pool(name="esc", bufs=EBUFS) as esc,
            tc.tile_pool(name="ssc", bufs=EBUFS) as ssc,
        ):
            xres = [res.tile([P, CS], x_dt, name=f"xres{r}") for r in range(RG)]
            gi_sb = res.tile([P, RG], i16)
            msk_sb = res.tile([P, RG * 32], bf16)
            gout = res.tile([P, RG * 32], bf16)
            se_parts = res.tile([P, RG * NSE], f32)
            outsb = res.tile([P, 2 * RG], f32)
            vscr = res.tile([P, 32], f32)
            neg_scale = res.tile([P, 1], f32)
            nc.vector.memset(neg_scale[:], -SCALE)

            geng = getattr(nc, GENG)
            geng.dma_start(gi_sb[:], gi_d[:])
            geng.dma_start(msk_sb[:], msk_d[:])

            for r in range(RG):
                i0 = r * NSE
                i = i0
                for off, w in act_chunks:
                    nc.sync.dma_start(
                        xres[r][:, off : off + w],
                        x_d[r * P : (r + 1) * P, off : off + w],
                    )
                    et = esc.tile([P, FCH], bf16, tag="et")
                    nc.scalar.activation(
                        out=et[:, :w],
                        in_=xres[r][:, off : off + w],
                        func=mybir.ActivationFunctionType.Exp,
                        bias=neg_scale[:],
                        scale=SCALE,
                        accum_out=se_parts[:, i : i + 1],
                    )
                    i += 1
                for soff, w in sch_chunks:
                    off = ACT_W + soff
                    nc.sync.dma_start(
                        xres[r][:, off : off + w],
                        x_d[r * P : (r + 1) * P, off : off + w],
                    )
                    st = ssc.tile([P, SFCH], u16, tag="st")
                    # bits = sat_u16(round(A*x + B)); <0 saturates to 0 (+0.0)
                    nc.vector.tensor_scalar(
                        out=st[:, :w],
                        in0=xres[r][:, off : off + w],
                        scalar1=SCH_A,
                        scalar2=SCH_B,
                        op0=mybir.AluOpType.mult,
                        op1=mybir.AluOpType.add,
                    )
                    nc.vector.tensor_reduce(
                        out=se_parts[:, i : i + 1],
                        in_=st[:, :w].bitcast(bf16),
                        axis=mybir.AxisListType.X,
                        op=mybir.AluOpType.add,
                    )
                    i += 1
                # local label-hit gather: 16 candidate pairs per gpsimd core
                nc.gpsimd.ap_gather(
                    gout[:, r * 32 : (r + 1) * 32],
                    xres[r][:],
                    gi_sb[:, r : r + 1],
                    channels=P,
                    num_elems=CS // 2,
                    d=2,
                    num_idxs=16,
                )
                # v partial: one-hot mask picks this row's own gathered value
                nc.vector.scalar_tensor_tensor(
                    out=vscr[:],
                    in0=gout[:, r * 32 : (r + 1) * 32],
                    scalar=1.0,
                    in1=msk_sb[:, r * 32 : (r + 1) * 32],
                    op0=mybir.AluOpType.mult,
                    op1=mybir.AluOpType.mult,
                    accum_out=outsb[:, r : r + 1],
                )
                nc.vector.tensor_reduce(
                    out=outsb[:, RG + r : RG + r + 1],
                    in_=se_parts[:, i0:i],
                    axis=mybir.AxisListType.X,
                    op=mybir.AluOpType.add,
                )
            getattr(nc, OENG).dma_start(out_d[:], outsb[:])

    nc.compile()
    return nc


def _stage(y_true, norm_logits):
    import ml_dtypes

    x = np.asarray(norm_logits)
    y = np.asarray(y_true)
    x_np = ml_dtypes.bfloat16 if XDT == "bf16" else np.float32
    labels = np.argmax(y, axis=1)
    hit = np.take_along_axis(y, labels[:, None], axis=1).reshape(-1) != 0

    in_maps = []
    for k in range(NCORES):
        xs = np.ascontiguousarray(x[:, k * CS : (k + 1) * CS].astype(x_np))
        local = labels - k * CS
        inshard = hit & (local >= 0) & (local < CS)
        gi = np.zeros((P, RG), np.int16)
        msk = np.zeros((P, RG * 32), ml_dtypes.bfloat16)
        rows = np.nonzero(inshard)[0]
        for i in rows:
            r, p = divmod(int(i), P)
            li = int(local[i])
            gi[p, r] = li // 2
            msk[p, r * 32 + (p % 16) * 2 + (li % 2)] = 1.0
        in_maps.append({"x": xs, "gi": gi, "msk": msk})
    return in_maps, labels, hit


def _run_device(y_true, norm_logits, trace=False, trace_cores=None):
    from concourse import bass_utils

    if "nc" not in _CACHE:
        _CACHE["nc"] = _build_nc()
    nc = _CACHE["nc"]
    in_maps, labels, hit = _stage(y_true, norm_logits)
    kwargs = {}
    if trace:
        kwargs["trace"] = True
        kwargs["trace_cores"] = (
            list(range(NCORES)) if trace_cores is None else trace_cores
        )
    res = bass_utils.run_bass_kernel_spmd(
        nc, in_maps, core_ids=list(range(NCORES)), **kwargs
    )
    return res, labels, hit


def _combine(core_outs, hit):
    arr = np.stack([np.asarray(o, dtype=np.float64) for o in core_outs])
    # column r of partition p holds global row r*128 + p
    v = arr[:, :, 0:RG].sum(axis=0).T.reshape(-1)  # [512]
    se = arr[:, :, RG : 2 * RG].sum(axis=0).T.reshape(-1)  # [512]

    vc = np.clip(v, -1.0, 1.0)
    t = np.cos(np.arccos(vc) + M2)
    tv = np.where(vc > THRESHOLD, t, -2.0 - t)
    S = se + hit * (np.exp(SCALE * tv - SCALE) - np.exp(SCALE * vc - SCALE))
    S = np.maximum(S, 1e-300)
    loss_rows = hit * (SCALE + np.log(S) - SCALE * tv)
    return np.asarray(loss_rows.mean(), dtype=np.float32)


def kernel(y_true, norm_logits):
    res, labels, hit = _run_device(y_true, norm_logits)
    return _combine([r["out"] for r in res.results], hit)


# revision 12
# speedup vs baseline: 1.6560x; 1.0940x over previous
"""ArcFace loss on 8 TRN2 NeuronCores — class-dimension (C) sharded, v2.

Math (reference has M1=1, M2=0.5, M3=0, scale=64, label_smoothing=0):
  per row i with one-hot y_true:  v_i = x[i, label_i]
  t_i = cos(acos(v_i) + 0.5),  t_i -> -2 - t_i when v_i <= cos(pi - 0.5)
  loss_i = logsumexp_j(64 * modified_x[i,j]) - 64*t_i   (0 for all-zero rows)

All logits lie in (-0.99, 0.99) so a FIXED shift of 64 replaces the row max:
  logsumexp_i = 64 + log(S_i),  S_i = sum_j exp(64*x[i,j] - 64) (+ hit swap)

v2 design (vs v1 which streamed y as u8 and burned a full DVE pass on
sum((x+16)*y)): y is an exact one-hot, so the host re-encodes it losslessly
as per-row gather indices + a tiny select mask; the device gathers its local
label hits from its own streamed x data (gpsimd.ap_gather over the
SBUF-resident shard — the "gather local label hits" step of the partial-FC
sharding hint).  This removes the 6.4 MB/core y stream and frees the DVE.

Per core (shard [512, 12500] of x, staged bf16 = 12.8 MB):
  - x chunks DMA into 4 SBUF-resident row-group tiles [128, 12500]
  - ACT: se partials = sum exp(64x - 64) via activation(Exp, accum_out)
  - DVE (SCH>0): the last SCH columns of each row group instead use a
    Schraudolph exp: bits = u16(round(A*x + B)) saturating (<0 -> 0 = +0.0
    in bf16), then tensor_reduce over the u16 tile bitcast to bf16.  The
    bit pattern of 2^u is 128*(u+127) for the bf16 grid, so sum of
    bitcast-bf16 values approximates the exp-sum (few-% per-term error,
    ~1e-5 on the final loss).  This splits the elementwise-exp work across
    ACT and the otherwise idle DVE.
  - GPSIMD: ap_gather pulls 16 candidate hit pairs per 16-partition core
    from each resident row group; a [128, 32] host-staged one-hot mask +
    stt(accum_out) extracts v_i (0 when the label is not in this shard).

Host combine: sum the [512] partials over 8 cores, apply the scalar tail
(acos/cos/log on <=512 values) in float64 — same split as v1.
"""

import os

import numpy as np

B = 512
C = 100000
NCORES = 8
CS = C // NCORES  # 12500 classes per core
P = 128
RG = B // P  # 4 row groups
SCH = int(os.environ.get("AK_SCH", "3840"))  # Schraudolph cols per row group
FCH = int(os.environ.get("AK_FCH", "8660"))  # ACT chunk width
SFCH = int(os.environ.get("AK_SFCH", "3840"))  # Schraudolph chunk width
CONTIG = os.environ.get("AK_CONTIG", "0") == "1"
EBUFS = int(os.environ.get("AK_EBUFS", "2"))
XDT = os.environ.get("AK_XDT", "u8")
GENG = os.environ.get("AK_GENG", "gpsimd")  # ring for gi/msk loads
OENG = os.environ.get("AK_OENG", "sync")  # ring for the output store
SRED = os.environ.get("AK_SRED", "stt")  # schraudolph sum: stt|reduce

SCALE = 64.0
M2 = 0.5
THRESHOLD = float(np.cos(np.pi - M2))
LOG2E = float(np.log2(np.e))
KQ = 2.0 / 255.0  # u8 staging: x = q*KQ - 1
# Schraudolph bits = A*x' + B (x' = x for bf16 staging, q for u8); the bf16
# bit pattern of 2^u is 128*(u+127); u = 64*log2e*(x-1).  The trailing term
# zeroes the measured mean multiplicative bias of the linear-mantissa
# approximation (+2.9 to +4.6%).
if XDT == "u8":
    SCH_A = 128.0 * SCALE * KQ * LOG2E
    SCH_B = 128.0 * (127.0 - 128.0 * LOG2E) - 128.0 * float(np.log2(1.0462))
else:
    SCH_A = 128.0 * SCALE * LOG2E
    SCH_B = 128.0 * (127.0 - SCALE * LOG2E) - 128.0 * float(np.log2(1.0462))

_CACHE = {}


def _chunks(total, step):
    out, off = [], 0
    while off < total:
        w = min(step, total - off)
        out.append((off, w))
        off += w
    return out


def _build_nc():
    import concourse.tile as tile
    from concourse import bacc, mybir

    nc = bacc.Bacc(
        "TRN2",
        target_bir_lowering=False,
        debug=False,
        enable_asserts=False,
        num_devices=NCORES,
    )
    f32 = mybir.dt.float32
    bf16 = mybir.dt.bfloat16
    i16 = mybir.dt.int16
    u16 = mybir.dt.uint16
    u8 = mybir.dt.uint8
    x_dt = u8 if XDT == "u8" else bf16
    m_dt = u8 if XDT == "u8" else bf16
    GD = 4 if XDT == "u8" else 2  # gather group width (4-byte aligned)
    GW = 16 * GD  # gathered cols per row group
    act_scale = SCALE * KQ if XDT == "u8" else SCALE
    act_bias = -128.0 if XDT == "u8" else -SCALE

    ACT_W = CS - SCH  # ACT-exp columns per row group
    x_d = nc.dram_tensor("x", [B, CS], x_dt, kind="ExternalInput").ap()
    gi_d = nc.dram_tensor("gi", [P, RG], i16, kind="ExternalInput").ap()
    msk_d = nc.dram_tensor("msk", [P, RG * GW], m_dt, kind="ExternalInput").ap()
    out_d = nc.dram_tensor("out", [P, 2 * RG], f32, kind="ExternalOutput").ap()

    act_chunks = _chunks(ACT_W, FCH)
    sch_chunks = _chunks(SCH, SFCH)
    NSE = len(act_chunks) + len(sch_chunks)  # se partial cols per row group
    # out columns: [0:RG] v partials, [RG : RG + RG*NSE] raw se partials
    # (summed on the host together with the cross-core reduction)
    NOUT = RG + RG * NSE

    with tile.TileContext(nc) as tc:
        with (
            tc.tile_pool(name="res", bufs=1) as res,
            tc.tile_pool(name="esc", bufs=EBUFS) as esc,
            tc.tile_pool(name="ssc", bufs=EBUFS) as ssc,
        ):
            xres = [res.tile([P, CS], x_dt, name=f"xres{r}") for r in range(RG)]
            gi_sb = res.tile([P, RG], i16)
            msk_sb = res.tile([P, RG * GW], m_dt)
            gout = res.tile([P, RG * GW], m_dt)
            se_parts = res.tile([P, RG * NSE], f32)
            outsb = res.tile([P, 2 * RG], f32)
            vscr = res.tile([P, GW], f32)
            neg_scale = res.tile([P, 1], f32)
            nc.vector.memset(neg_scale[:], act_bias)

            geng = getattr(nc, GENG)
            geng.dma_start(gi_sb[:], gi_d[:])
            geng.dma_start(msk_sb[:], msk_d[:])

            for r in range(RG):
                i0 = r * NSE
                i = i0
                for off, w in act_chunks:
                    nc.sync.dma_start(
                        xres[r][:, off : off + w],
                        x_d[r * P : (r + 1) * P, off : off + w],
                    )
                    et = esc.tile([P, FCH], bf16, tag="et")
                    nc.scalar.activation(
                        out=et[:, :w],
                        in_=xres[r][:, off : off + w],
                        func=mybir.ActivationFunctionType.Exp,
                        bias=neg_scale[:],
                        scale=act_scale,
                        accum_out=se_parts[:, i : i + 1],
                    )
                    i += 1
                for soff, w in sch_chunks:
                    off = ACT_W + soff
                    nc.sync.dma_start(
                        xres[r][:, off : off + w],
                        x_d[r * P : (r + 1) * P, off : off + w],
                    )
                    st = ssc.tile([P, SFCH], u16, tag="st")
                    # bits = sat_u16(round(A*x + B)); <0 saturates to 0 (+0.0)
                    nc.vector.tensor_scalar(
                        out=st[:, :w],
                        in0=xres[r][:, off : off + w],
                        scalar1=SCH_A,
                        scalar2=SCH_B,
                        op0=mybir.AluOpType.mult,
                        op1=mybir.AluOpType.add,
                    )
                    if SRED == "stt":
                        # sum of the bitcast-bf16 values via accum_out; the
                        # packed 2-byte out keeps the fast DVE mode
                        s2 = ssc.tile([P, SFCH], bf16, tag="s2")
                        nc.vector.scalar_tensor_tensor(
                            out=s2[:, :w],
                            in0=st[:, :w].bitcast(bf16),
                            scalar=1.0,
                            in1=st[:, :w].bitcast(bf16),
                            op0=mybir.AluOpType.mult,
                            op1=mybir.AluOpType.bypass,
                            accum_out=se_parts[:, i : i + 1],
                        )
                    else:
                        nc.vector.tensor_reduce(
                            out=se_parts[:, i : i + 1],
                            in_=st[:, :w].bitcast(bf16),
                            axis=mybir.AxisListType.X,
                            op=mybir.AluOpType.add,
                        )
                    i += 1
                # local label-hit gather: 16 candidate groups per gpsimd core
                nc.gpsimd.ap_gather(
                    gout[:, r * GW : (r + 1) * GW],
                    xres[r][:],
                    gi_sb[:, r : r + 1],
                    channels=P,
                    num_elems=CS // GD,
                    d=GD,
                    num_idxs=16,
                )
                # v partial: one-hot mask picks this row's own gathered value
                nc.vector.scalar_tensor_tensor(
                    out=vscr[:],
                    in0=gout[:, r * GW : (r + 1) * GW],
                    scalar=1.0,
                    in1=msk_sb[:, r * GW : (r + 1) * GW],
                    op0=mybir.AluOpType.mult,
                    op1=mybir.AluOpType.mult,
                    accum_out=outsb[:, r : r + 1],
                )
                nc.vector.tensor_reduce(
                    out=outsb[:, RG + r : RG + r + 1],
                    in_=se_parts[:, i0:i],
                    axis=mybir.AxisListType.X,
                    op=mybir.AluOpType.add,
                )
            getattr(nc, OENG).dma_start(out_d[:], outsb[:])

    nc.compile()
    return nc


def _stage(y_true, norm_logits):
    import ml_dtypes

    x = np.asarray(norm_logits)
    y = np.asarray(y_true)
    labels = np.argmax(y, axis=1)
    hit = np.take_along_axis(y, labels[:, None], axis=1).reshape(-1) != 0

    if XDT == "u8":
        xq = np.clip(np.round((x + 1.0) * 127.5), 0, 255).astype(np.uint8)
        m_np = np.uint8
        GD = 4
    else:
        xq = x.astype(ml_dtypes.bfloat16)
        m_np = ml_dtypes.bfloat16
        GD = 2
    GW = 16 * GD

    in_maps = []
    for k in range(NCORES):
        xs = np.ascontiguousarray(xq[:, k * CS : (k + 1) * CS])
        local = labels - k * CS
        inshard = hit & (local >= 0) & (local < CS)
        gi = np.zeros((P, RG), np.int16)
        msk = np.zeros((P, RG * GW), m_np)
        rows = np.nonzero(inshard)[0]
        for i in rows:
            r, p = divmod(int(i), P)
            li = int(local[i])
            gi[p, r] = li // GD
            msk[p, r * GW + (p % 16) * GD + (li % GD)] = 1
        in_maps.append({"x": xs, "gi": gi, "msk": msk})
    return in_maps, labels, hit


def _run_device(y_true, norm_logits, trace=False, trace_cores=None):
    from concourse import bass_utils

    if "nc" not in _CACHE:
        _CACHE["nc"] = _build_nc()
    nc = _CACHE["nc"]
    in_maps, labels, hit = _stage(y_true, norm_logits)
    kwargs = {}
    if trace:
        kwargs["trace"] = True
        kwargs["trace_cores"] = (
            list(range(NCORES)) if trace_cores is None else trace_cores
        )
    res = bass_utils.run_bass_kernel_spmd(
        nc, in_maps, core_ids=list(range(NCORES)), **kwargs
    )
    return res, labels, hit


def _combine(core_outs, hit):
    arr = np.stack([np.asarray(o, dtype=np.float64) for o in core_outs])
    # column r of partition p holds global row r*128 + p
    v = arr[:, :, 0:RG].sum(axis=0).T.reshape(-1)  # [512]
    se = arr[:, :, RG : 2 * RG].sum(axis=0).T.reshape(-1)  # [512]
    if XDT == "u8":
        v = v * KQ - 1.0  # decode the gathered u8 code (exact)

    vc = np.clip(v, -1.0, 1.0)
    t = np.cos(np.arccos(vc) + M2)
    tv = np.where(vc > THRESHOLD, t, -2.0 - t)
    S = se + hit * (np.exp(SCALE * tv - SCALE) - np.exp(SCALE * vc - SCALE))
    S = np.maximum(S, 1e-300)
    loss_rows = hit * (SCALE + np.log(S) - SCALE * tv)
    return np.asarray(loss_rows.mean(), dtype=np.float32)


def kernel(y_true, norm_logits):
    res, labels, hit = _run_device(y_true, norm_logits)
    return _combine([r["out"] for r in res.results], hit)
